# revision 1
# baseline (speedup 1.0000x reference)
"""Trainium2 Bass kernel for ContrastiveMultiTaskLoss.

Computes, on 8 NeuronCores (SPMD, no collectives):
  loss = 1.0*mse(price) + 0.5*mse(change) + 0.3*bce(crit)
       + 0.1 * NT-Xent(z1, z2, temp=0.1)

Strategy: every core receives the full z1/z2 ([8192,256] f32) plus a
row-block of queries zq ([2048,256]) and small per-core slices for the
positive-pair / supervised partial sums.  Each core:
  - normalizes all 16384 embedding rows (norms via bn_stats on DVE,
    rnorm = exp(-0.5*ln(n2)) on ACT), casts to bf16, and PE-transposes
    them into a resident SBUF layout znT[K=2][128, 16384]
  - normalizes+transposes its 2048 query rows identically (bit-identical
    values, so the sim diagonal is exp(10*||znq_bf16||^2) ~= e^10)
  - computes its [2048, 16384] sim slice with bf16 matmuls (K=256 as 2
    accumulating steps, N=512 per PSUM bank), exponentiates each
    [128,2048] PSUM tile in place on the scalar engine (Exp, scale=10)
    with accum_out producing row sums directly
  - subtracts e^10 (self-sim), takes Ln, accumulates log-sum-exp partials
  - computes positive-pair dots and supervised loss partials
The host sums the 8 [128,8] partial tensors and applies the loss weights.

All activation funcs used (Exp/Ln/Square/Identity/Copy/Relu/Abs) live in
the single ACT table `natural_log_exp_and_others`; _pin_act_tables makes
the greedy table-selection pass pick it so no mid-kernel table reloads
are emitted.
"""

import math

import numpy as np

import concourse.bass as bass
import concourse.mybir as mybir
import concourse.tile as tile
from concourse import bacc
from concourse.bass_utils import run_bass_kernel_spmd
from concourse.masks import make_identity

F32 = mybir.dt.float32
BF16 = mybir.dt.bfloat16
AF = mybir.ActivationFunctionType

N_CORES = 8
D = 256
KH = 2           # K halves (D = 2*128)
GCOLS = 2048     # columns per sim-group (4 PSUM banks of 512 f32)
ITEMP = 10.0     # 1/temperature
W_PRICE, W_CHANGE, W_CRIT = 1.0, 0.5, 0.3
SSL_WEIGHT = 0.1
PAD_MM = 0   # zero-matmuls per round to keep PE warm (0 = off)
MUL_ENGINE = "alt"   # "dve" | "pool" | "alt" - engine for zn scale-muls
POOL_BUFS = (3, 3, 4)  # loads, stage, small
EXP_SELF = float(np.exp(10.0).astype(np.float64))  # exp(ITEMP * ||zn||^2)
ACT_TABLE = "natural_log_exp_and_others"


class Cfg:
    def __init__(self, n):
        self.n = n                       # rows in z1 (= rows in z2)
        self.two_n = 2 * n
        self.rows_q = 2 * n // N_CORES   # query rows per core
        self.n_rowtiles = self.rows_q // 128
        self.n_groups = self.two_n // GCOLS
        self.pos_rows = n // N_CORES     # pos-pair rows per core
        self.pa = self.pos_rows // 128   # chunks of 128
        self.sup_rows = n // N_CORES
        self.sa = self.sup_rows // 128


FULL = Cfg(8192)


def _pin_act_tables(nc):
    """Make the act-table pass choose the one table containing all our
    funcs (ids/contents untouched for the chosen table: we only remove our
    funcs from the *other* tables so greedy selection can't pick them)."""
    import concourse.hw_specs as hw_specs
    tabs = hw_specs.get_activation_tables(nc.m.arch)
    ours = set(tabs[ACT_TABLE])
    for name, funcs in tabs.items():
        if name != ACT_TABLE:
            funcs -= ours


def build_program(cfg, repeat=1):
    nc = bacc.Bacc("TRN2", target_bir_lowering=False, debug=False,
                   num_devices=N_CORES)
    z1_ext = nc.dram_tensor("z1", [cfg.n, D], F32, kind="ExternalInput")
    z2_ext = nc.dram_tensor("z2", [cfg.n, D], F32, kind="ExternalInput")
    zq_ext = nc.dram_tensor("zq", [cfg.rows_q, D], F32, kind="ExternalInput")
    zp1_ext = nc.dram_tensor("zp1", [cfg.pos_rows, D], F32, kind="ExternalInput")
    zp2_ext = nc.dram_tensor("zp2", [cfg.pos_rows, D], F32, kind="ExternalInput")
    sup_ext = nc.dram_tensor("sup", [6, cfg.sup_rows], F32, kind="ExternalInput")
    part_ext = nc.dram_tensor("partials", [128, 8], F32, kind="ExternalOutput")

    with tile.TileContext(nc) as tc:
        for _ in range(repeat):
            _emit(nc, tc, cfg, z1_ext, z2_ext, zq_ext, zp1_ext, zp2_ext,
                  sup_ext, part_ext)
    _pin_act_tables(nc)
    nc.compile()
    return nc


def _emit(nc, tc, cfg, z1_ext, z2_ext, zq_ext, zp1_ext, zp2_ext,
          sup_ext, part_ext):
    from contextlib import ExitStack
    ctx = ExitStack()
    with ctx:
        singles = ctx.enter_context(tc.tile_pool(name="singles", bufs=1))
        loads = ctx.enter_context(tc.tile_pool(name="loads", bufs=POOL_BUFS[0]))
        stage = ctx.enter_context(tc.tile_pool(name="stage", bufs=POOL_BUFS[1]))
        small = ctx.enter_context(tc.tile_pool(name="small", bufs=POOL_BUFS[2]))

        ident = singles.tile([128, 128], BF16, tag="ident")
        make_identity(nc, ident[:])

        _bias_tiles = {}

        def bias_const(val):
            if val not in _bias_tiles:
                t = singles.tile([128, 1], F32, tag=f"bias{len(_bias_tiles)}",
                                 name=f"bias{len(_bias_tiles)}")
                nc.vector.memset(t[:], val)
                _bias_tiles[val] = t
            return _bias_tiles[val][:]

        partials = singles.tile([128, 8], F32, tag="partials")
        nc.vector.memset(partials[:, 5:8], 0.0)
        # zero stationary for PE-pacing pad matmuls (adds 0 to PSUM):
        # keeps PE continuously busy so it stays at full clock (the HW HAM
        # window tolerates the small gap; the cost model does not)
        zeros = singles.tile([128, 128], BF16, tag="zeros")
        nc.vector.memset(zeros[:], 0.0)

        # resident transposed bf16 embeddings: znt[h][g] = [128, GCOLS]
        znt = [[singles.tile([128, GCOLS], BF16, tag=f"znt_{h}_{g}",
                             name=f"znt_{h}_{g}")
                for g in range(cfg.n_groups)] for h in range(KH)]
        # resident transposed bf16 queries: znqt[h] = [128, rows_q]
        znqt = [singles.tile([128, cfg.rows_q], BF16, tag=f"znqt_{h}",
                             name=f"znqt_{h}")
                for h in range(KH)]
        loghold = singles.tile([128, cfg.n_rowtiles], F32, tag="loghold")
        # exp(ITEMP * ||znq_bf16||^2) per query rowtile (exact diag values)
        d2q = singles.tile([128, cfg.n_rowtiles], F32, tag="d2q")
        expdq = singles.tile([128, cfg.n_rowtiles], F32, tag="expdq")
        # accumulated exp row sums, one col per (rowtile, group)
        acc_all = singles.tile([128, cfg.n_rowtiles, cfg.n_groups], F32,
                               tag="acc_all")

        # ---------------- prologue: normalize + transpose -------------
        bigtile_idx = [0]

        def norm_cast(big_rows_ap, A, out_bf):
            """Load [128, A, 256] f32 rows; write normalized bf16 to out_bf.

            norms via bn_stats/bn_aggr (DVE), rnorm = exp(-0.5 ln(256*n2m))
            computed on ACT with the ln(256) folded into the exp bias."""
            zbig = loads.tile([128, A, D], F32, tag="zbig")
            nc.sync.dma_start(out=zbig[:], in_=big_rows_ap)
            stats = small.tile([128, A, 6], F32, tag="stats")
            mv = small.tile([128, A, 2], F32, tag="mv")
            for a in range(A):
                nc.vector.bn_stats(out=stats[:, a, :], in_=zbig[:, a, :])
                nc.vector.bn_aggr(out=mv[:, a, :], in_=stats[:, a, :])
            m2 = small.tile([128, A], F32, tag="m2")
            nc.vector.tensor_mul(m2[:], mv[:, :, 0], mv[:, :, 0])
            n2m = small.tile([128, A], F32, tag="n2m")
            nc.vector.tensor_add(n2m[:], m2[:], mv[:, :, 1])
            lnn = small.tile([128, A], F32, tag="lnn")
            nc.scalar.activation(out=lnn[:], in_=n2m[:], func=AF.Ln)
            rn = small.tile([128, A], F32, tag="rn")
            nc.scalar.activation(out=rn[:], in_=lnn[:], func=AF.Exp,
                                 scale=-0.5, bias=bias_const(-0.5 * math.log(D)))
            if MUL_ENGINE == "dve":
                eng = nc.vector
            elif MUL_ENGINE == "pool":
                eng = nc.gpsimd
            else:
                eng = nc.vector if bigtile_idx[0] % 2 == 0 else nc.gpsimd
            bigtile_idx[0] += 1
            for a in range(A):
                eng.tensor_scalar_mul(out_bf[:, a, :], zbig[:, a, :],
                                      rn[:, a:a + 1])

        def transpose_block(zn_bf, A, dest_fn, psum_pool):
            """PE-transpose [128,256] chunks; batch PSUM->SBUF copies.

            Shares the main-loop PSUM slots (tag "mp") so prologue and main
            loop can interleave without address-reuse serialization.

            dest_fn(h) -> (tile, col_offset) for the A*128-wide block."""
            for h in range(KH):
                pt = psum_pool.tile([128, A * 128], BF16, tag="mp", name="pt")
                for a in range(A):
                    nc.tensor.transpose(pt[:, a * 128:(a + 1) * 128],
                                        zn_bf[:, a, h * 128:(h + 1) * 128],
                                        ident[:])
                dst, off = dest_fn(h)
                nc.vector.tensor_copy(dst[:, off:off + A * 128], pt[:])

        with tc.tile_pool(name="mpsum", bufs=2, space="PSUM") as mpsum:
            # queries first: the main loop depends on them for every group
            zqr = zq_ext.ap().rearrange("(c p) d -> p c d", p=128)
            nqchunks = cfg.rows_q // 128
            for start in range(0, nqchunks, 8):
                A = min(8, nqchunks - start)
                znb = stage.tile([128, A, D], BF16, tag="znb")
                norm_cast(zqr[:, start:start + A, :], A, znb)
                # exact self-sim ||znq_bf16||^2 via bn stats on the bf16 tile
                qstats = small.tile([128, A, 6], F32, tag="qstats")
                qmv = small.tile([128, A, 2], F32, tag="qmv")
                for a in range(A):
                    nc.vector.bn_stats(out=qstats[:, a, :], in_=znb[:, a, :])
                    nc.vector.bn_aggr(out=qmv[:, a, :], in_=qstats[:, a, :])
                qm2 = small.tile([128, A], F32, tag="qm2")
                nc.vector.tensor_mul(qm2[:], qmv[:, :, 0], qmv[:, :, 0])
                nc.vector.tensor_add(d2q[:, start:start + A], qm2[:],
                                     qmv[:, :, 1])
                transpose_block(znb, A,
                                lambda h, s=start: (znqt[h], s * 128),
                                mpsum)
            # expdq = exp(ITEMP * D * d2q_mean)
            nc.scalar.activation(out=expdq[:], in_=d2q[:], func=AF.Exp,
                                 scale=ITEMP * D)

            # keys: z1 then z2, with each group's main-loop rounds
            # emitted as soon as the group's transposed keys are complete
            # (PSUM slots are allocation-ordered, so interleaving emission
            # is what lets sim matmuls overlap the rest of the prologue)
            def main_rounds(g):
                for m in range(cfg.n_rowtiles):
                    pt = mpsum.tile([128, GCOLS], F32, tag="mp", name="mp")
                    for h in range(KH):
                        for j in range(GCOLS // 512):
                            last = h == KH - 1
                            stop = last and (PAD_MM == 0 or j > 0)
                            nc.tensor.matmul(
                                pt[:, j * 512:(j + 1) * 512],
                                lhsT=znqt[h][:, m * 128:(m + 1) * 128],
                                rhs=znt[h][g][:, j * 512:(j + 1) * 512],
                                start=(h == 0), stop=stop)
                    for pad in range(PAD_MM):
                        nc.tensor.matmul(
                            pt[:, 0:512], lhsT=zeros[:],
                            rhs=znt[0][g][:, 0:512],
                            start=False, stop=(pad == PAD_MM - 1))
                    nc.scalar.activation(out=pt[:], in_=pt[:], func=AF.Exp,
                                         scale=ITEMP,
                                         accum_out=acc_all[:, m, g:g + 1])

            chunks_done = 0
            groups_emitted = 0
            for zi, z_ext in enumerate((z1_ext, z2_ext)):
                zr = z_ext.ap().rearrange("(c p) d -> p c d", p=128)
                nchunks = cfg.n // 128
                for start in range(0, nchunks, 8):
                    A = min(8, nchunks - start)
                    znb = stage.tile([128, A, D], BF16, tag="znb")
                    norm_cast(zr[:, start:start + A, :], A, znb)
                    base_chunk = zi * nchunks + start
                    g, c = divmod(base_chunk, GCOLS // 128)
                    transpose_block(znb, A,
                                    lambda h, g=g, c=c: (znt[h][g], c * 128),
                                    mpsum)
                    chunks_done += A
                    while (groups_emitted + 1) * (GCOLS // 128) <= chunks_done:
                        main_rounds(groups_emitted)
                        groups_emitted += 1
            assert groups_emitted == cfg.n_groups

            # ---- positive pairs + supervised (tiny) ----
            _emit_pos_sup(nc, tc, cfg, zp1_ext, zp2_ext, sup_ext,
                          partials, loads, stage, small, bias_const)

        # batched epilogue: row sums per rowtile, minus exp(self), log
        rs_all = small.tile([128, cfg.n_rowtiles], F32, tag="rs_all")
        nc.vector.tensor_reduce(out=rs_all[:], in_=acc_all[:],
                                axis=mybir.AxisListType.X,
                                op=mybir.AluOpType.add)
        rsc_all = small.tile([128, cfg.n_rowtiles], F32, tag="rsc_all")
        nc.vector.tensor_sub(rsc_all[:], rs_all[:], expdq[:])
        nc.scalar.activation(out=loghold[:], in_=rsc_all[:], func=AF.Ln)
        lhdump = small.tile([128, cfg.n_rowtiles], F32, tag="lhdump")
        nc.scalar.activation(out=lhdump[:], in_=loghold[:], func=AF.Identity,
                             accum_out=partials[:, 0:1])
        nc.sync.dma_start(out=part_ext[:], in_=partials[:])


def _emit_pos_sup(nc, tc, cfg, zp1_ext, zp2_ext, sup_ext, partials,
                  loads, stage, small, bias_const):
    A = cfg.pa
    # --- positive pair partial: sum over rows of zn1 . zn2 (unscaled) ---
    zp1r = zp1_ext.ap().rearrange("(a p) d -> p a d", p=128)
    zp2r = zp2_ext.ap().rearrange("(a p) d -> p a d", p=128)
    p1 = loads.tile([128, A, D], F32, tag="p1", bufs=1)
    p2 = loads.tile([128, A, D], F32, tag="p2", bufs=1)
    nc.sync.dma_start(out=p1[:], in_=zp1r)
    nc.sync.dma_start(out=p2[:], in_=zp2r)
    prod = stage.tile([128, A, D], F32, tag="prod", bufs=1)
    nc.vector.tensor_mul(prod[:], p1[:], p2[:])
    # bn_stats means: n2{a,b} = D*(var+mean^2); dots = D*mean(prod)
    stats = small.tile([128, 3, A, 6], F32, tag="pstats")
    mv = small.tile([128, 3, A, 2], F32, tag="pmv")
    for i, src in enumerate((p1, p2, prod)):
        for a in range(A):
            nc.vector.bn_stats(out=stats[:, i, a, :], in_=src[:, a, :])
            nc.vector.bn_aggr(out=mv[:, i, a, :], in_=stats[:, i, a, :])
    m2 = small.tile([128, 2, A], F32, tag="pm2")
    nc.vector.tensor_mul(m2[:], mv[:, 0:2, :, 0], mv[:, 0:2, :, 0])
    n2ab = small.tile([128, 2, A], F32, tag="n2ab")
    nc.vector.tensor_add(n2ab[:], m2[:], mv[:, 0:2, :, 1])
    # rnorm product: exp(-0.5*(ln(n2a*D) + ln(n2b*D)))
    lnab = small.tile([128, 2, A], F32, tag="lnab")
    nc.scalar.activation(out=lnab[:], in_=n2ab[:], func=AF.Ln)
    lnsum = small.tile([128, A], F32, tag="lnsum")
    nc.vector.tensor_add(lnsum[:], lnab[:, 0, :], lnab[:, 1, :])
    rp = small.tile([128, A], F32, tag="rp")
    nc.scalar.activation(out=rp[:], in_=lnsum[:], func=AF.Exp,
                         scale=-0.5, bias=bias_const(-math.log(D)))
    # pos = dots * rp = (D*mean(prod)) * rp
    pos = small.tile([128, A], F32, tag="pos")
    nc.vector.tensor_mul(pos[:], mv[:, 2, :, 0], rp[:])
    pdump = small.tile([128, A], F32, tag="pdump")
    # accumulate D * sum(pos)
    nc.scalar.activation(out=pdump[:], in_=pos[:], func=AF.Identity,
                         scale=float(D), accum_out=partials[:, 1:2])

    # --- supervised partials ---
    S = cfg.sa
    supr = sup_ext.ap().rearrange("s (p a) -> p s a", p=128)
    sup = loads.tile([128, 6, S], F32, tag="sup", bufs=1)
    nc.sync.dma_start(out=sup[:], in_=supr)
    d8 = small.tile([128, S], F32, tag="d8")
    sdump = small.tile([128, S], F32, tag="sdump")
    nc.vector.tensor_sub(d8[:], sup[:, 0, :], sup[:, 1, :])
    nc.scalar.activation(out=sdump[:], in_=d8[:], func=AF.Square,
                         accum_out=partials[:, 2:3])
    d8b = small.tile([128, S], F32, tag="d8b")
    nc.vector.tensor_sub(d8b[:], sup[:, 2, :], sup[:, 3, :])
    nc.scalar.activation(out=sdump[:], in_=d8b[:], func=AF.Square,
                         accum_out=partials[:, 3:4])
    # bce: relu(x) - x*t + ln(1 + exp(-|x|))
    x_ap = sup[:, 4, :]
    t_ap = sup[:, 5, :]
    r8 = small.tile([128, S], F32, tag="r8")
    nc.scalar.activation(out=r8[:], in_=x_ap, func=AF.Relu)
    a8 = small.tile([128, S], F32, tag="a8")
    nc.scalar.activation(out=a8[:], in_=x_ap, func=AF.Abs)
    e8 = small.tile([128, S], F32, tag="e8")
    nc.scalar.activation(out=e8[:], in_=a8[:], func=AF.Exp, scale=-1.0)
    l8 = small.tile([128, S], F32, tag="l8")
    nc.scalar.activation(out=l8[:], in_=e8[:], func=AF.Ln, bias=1.0)
    xt8 = small.tile([128, S], F32, tag="xt8")
    nc.vector.tensor_mul(xt8[:], x_ap, t_ap)
    s1 = small.tile([128, S], F32, tag="s1")
    nc.vector.tensor_add(s1[:], r8[:], l8[:])
    s2 = small.tile([128, S], F32, tag="s2")
    nc.vector.tensor_sub(s2[:], s1[:], xt8[:])
    nc.scalar.activation(out=sdump[:], in_=s2[:], func=AF.Identity,
                         accum_out=partials[:, 4:5])


def make_in_maps(cfg, price_pred, price_target, change_pred, change_target,
                 criticality_pred, criticality_target, z1, z2):
    z1 = np.ascontiguousarray(np.asarray(z1, dtype=np.float32))
    z2 = np.ascontiguousarray(np.asarray(z2, dtype=np.float32))
    sups = [np.asarray(a, dtype=np.float32).reshape(-1) for a in
            (price_pred, price_target, change_pred, change_target,
             criticality_pred, criticality_target)]
    in_maps = []
    rq = cfg.rows_q
    pr = cfg.pos_rows
    for c in range(N_CORES):
        qstart = c * rq
        if qstart < cfg.n:
            zq = z1[qstart:qstart + rq]
        else:
            zq = z2[qstart - cfg.n:qstart - cfg.n + rq]
        sl = slice(c * pr, (c + 1) * pr)
        sup = np.stack([s[c * cfg.sup_rows:(c + 1) * cfg.sup_rows]
                        for s in sups])
        in_maps.append({
            "z1": z1, "z2": z2,
            "zq": np.ascontiguousarray(zq),
            "zp1": np.ascontiguousarray(z1[sl]),
            "zp2": np.ascontiguousarray(z2[sl]),
            "sup": np.ascontiguousarray(sup),
        })
    return in_maps


def combine(cfg, results):
    cols = np.zeros(8, dtype=np.float64)
    for r in results:
        cols += r["partials"].astype(np.float64).sum(axis=0)
    slog, sdot, sprice, schange, scrit = cols[0], cols[1], cols[2], cols[3], cols[4]
    n = float(cfg.n)
    ssl = (slog - 2.0 * ITEMP * sdot) / (2.0 * n)
    supervised = (W_PRICE * sprice + W_CHANGE * schange + W_CRIT * scrit) / n
    return np.float32(supervised + SSL_WEIGHT * ssl)


_compiled = {}


def _get_program(repeat=1):
    key = repeat
    if key not in _compiled:
        _compiled[key] = build_program(FULL, repeat=repeat)
    return _compiled[key]


def kernel(**inputs):
    nc = _get_program()
    in_maps = make_in_maps(FULL, **inputs)
    res = run_bass_kernel_spmd(nc, in_maps, list(range(N_CORES)))
    return combine(FULL, res.results)



# revision 2
# speedup vs baseline: 1800.7803x; 1800.7803x over previous
"""Trainium2 Bass kernel for ContrastiveMultiTaskLoss — v2.

Computes, on 8 NeuronCores (SPMD, no collectives):
  loss = 1.0*mse(price) + 0.5*mse(change) + 0.3*bce(crit)
       + 0.1 * NT-Xent(z1, z2, temp=0.1)

v2 strategy (vs v1 which computed every core's full [2048, 16384] sim
slice): exploit sim-matrix symmetry so each unordered 128x128 tile pair
is computed ONCE globally, halving both the PE matmul and the ACT exp
work, and use fp8e4 DoubleRow matmuls (2x PE throughput).

Work assignment is made SPMD-uniform via a rotated key layout: core c
receives zcat = concat(z1,z2)[(2048c + 0..10239) % 16384] (rows then
wrap), so its 16 query rowtiles are local tiles 0..15 and query rowtile
t multiplies against local columns [128t, 128t+8320) — i.e. shifts
r = 0..64 to its right (mod 128 globally). Coverage: each unordered
off-diagonal pair {A,B} with tile distance d=(B-A)%128 in 1..63 is
computed exactly once (by the core owning the left rowtile); d=64 pairs
are computed twice but contribute only via the row path; diagonal tiles
contribute only via the row path. Column(=transposed-row) sums are
recovered with tiny lhsT=exp_tile, rhs=ones PE matmuls and accumulated
into an SBUF [128, 80] tensor; the final ln + cross-core assembly of
per-row denominators happens on the host in float64.

Embeddings are normalized, scaled by 4 (avoids fp8 subnormals), cast to
fp8e4 and PE-transposed into a resident znt8 [128, 2, 10240] layout
(k-subtile-major for DoubleRow). Exp runs on ACT with scale 1/(16*temp),
writing bf16 exp tiles to SBUF (for the colsum matmuls) and row sums via
accum_out. The self-sim diagonal exp(10*||q8||^2) is reproduced exactly
from bn_stats over the fp8 values and subtracted on device.

All activation funcs used (Exp/Ln/Square/Identity/Relu/Abs) live in the
single ACT table `natural_log_exp_and_others`; _pin_act_tables makes the
greedy table-selection pass pick it so no mid-kernel table reloads.
"""

import math

import numpy as np

import concourse.bass as bass
import concourse.mybir as mybir
import concourse.tile as tile
from concourse import bacc
from concourse.bass_utils import run_bass_kernel_spmd
from concourse.masks import make_identity

F32 = mybir.dt.float32
BF16 = mybir.dt.bfloat16
FP8 = mybir.dt.float8e4
AF = mybir.ActivationFunctionType

N_CORES = 8
D = 256
KH = 2                 # K halves (D = 2*128)
N = 8192
TWO_N = 2 * N
NT = TWO_N // 128      # 128 global rowtiles
QT = NT // N_CORES     # 16 query rowtiles per core
SHIFTS = NT // 2 + 1   # 65 coltile shifts (r = 0..64)
SPAN = SHIFTS * 128    # 8320 columns per query rowtile
KEYTILES = QT + SHIFTS - 1   # 80 local coltiles used per core
KEYROWS = KEYTILES * 128     # 10240 zcat rows per core
CHUNKS = (1536, 1536, 1536, 1536, 1536, 640)  # per-rowtile span chunking
LCH = 1024             # prologue load-chunk rows (8 tiles)
NLC = KEYROWS // LCH   # 10 load chunks
ITEMP = 10.0
SCALE_Q = 4.0          # pre-fp8 scale; exp scale becomes ITEMP/SCALE_Q^2
EXP_SCALE = ITEMP / (SCALE_Q * SCALE_Q)
W_PRICE, W_CHANGE, W_CRIT = 1.0, 0.5, 0.3
SSL_WEIGHT = 0.1
ACT_TABLE = "natural_log_exp_and_others"
DR = mybir.MatmulPerfMode.DoubleRow


def _chunks_for(t):
    out = []
    c0 = 128 * t
    for ch in CHUNKS:
        out.append((c0, ch))
        c0 += ch
    assert c0 == 128 * t + SPAN
    return out


class Cfg:
    def __init__(self, n):
        self.n = n
        self.pos_rows = n // N_CORES     # pos-pair rows per core
        self.pa = self.pos_rows // 128
        self.sup_rows = n // N_CORES
        self.sa = self.sup_rows // 128


FULL = Cfg(N)


def _pin_act_tables(nc):
    import concourse.hw_specs as hw_specs
    tabs = hw_specs.get_activation_tables(nc.m.arch)
    ours = set(tabs[ACT_TABLE])
    for name, funcs in tabs.items():
        if name != ACT_TABLE:
            funcs -= ours


def build_program(cfg, repeat=1):
    nc = bacc.Bacc("TRN2", target_bir_lowering=False, debug=False,
                   num_devices=N_CORES)
    zc_ext = nc.dram_tensor("zcat", [KEYROWS, D], F32, kind="ExternalInput")
    zp1_ext = nc.dram_tensor("zp1", [cfg.pos_rows, D], F32, kind="ExternalInput")
    zp2_ext = nc.dram_tensor("zp2", [cfg.pos_rows, D], F32, kind="ExternalInput")
    sup_ext = nc.dram_tensor("sup", [6, cfg.sup_rows], F32, kind="ExternalInput")
    part_ext = nc.dram_tensor("partials", [128, 8], F32, kind="ExternalOutput")
    rowadj_ext = nc.dram_tensor("rowadj", [128, QT], F32, kind="ExternalOutput")
    colacc_ext = nc.dram_tensor("colacc", [128, KEYTILES], F32,
                                kind="ExternalOutput")

    with tile.TileContext(nc) as tc:
        for _ in range(repeat):
            _emit(nc, tc, cfg, zc_ext, zp1_ext, zp2_ext, sup_ext,
                  part_ext, rowadj_ext, colacc_ext)
    _pin_act_tables(nc)
    nc.compile()
    return nc


def _emit(nc, tc, cfg, zc_ext, zp1_ext, zp2_ext, sup_ext,
          part_ext, rowadj_ext, colacc_ext):
    from contextlib import ExitStack
    ctx = ExitStack()
    with ctx:
        singles = ctx.enter_context(tc.tile_pool(name="singles", bufs=1))
        loads = ctx.enter_context(tc.tile_pool(name="loads", bufs=4))
        stage = ctx.enter_context(tc.tile_pool(name="stage", bufs=3))
        small = ctx.enter_context(tc.tile_pool(name="small", bufs=4))
        texps = ctx.enter_context(tc.tile_pool(name="texps", bufs=2))

        ident = singles.tile([128, 128], BF16, tag="ident")
        make_identity(nc, ident[:])
        ones = singles.tile([128, 1], BF16, tag="ones")
        nc.vector.memset(ones[:], 1.0)

        _bias_tiles = {}

        def bias_const(val):
            if val not in _bias_tiles:
                t = singles.tile([128, 1], F32, tag=f"bias{len(_bias_tiles)}",
                                 name=f"bias{len(_bias_tiles)}")
                nc.vector.memset(t[:], val)
                _bias_tiles[val] = t
            return _bias_tiles[val][:]

        partials = singles.tile([128, 8], F32, tag="partials")
        nc.vector.memset(partials[:, 0:1], 0.0)
        nc.vector.memset(partials[:, 5:8], 0.0)

        # resident transposed fp8 keys: [128, 2(k-half), KEYROWS]
        znt8 = singles.tile([128, KH, KEYROWS], FP8, tag="znt8")
        # per-query ||4*q8||^2 / D (mean^2 + var from bn_stats) and its exp
        d2q = singles.tile([128, QT], F32, tag="d2q")
        expdq = singles.tile([128, QT], F32, tag="expdq")
        # exp row-sum partials, one col per (rowtile, chunk)
        accq = singles.tile([128, QT, len(CHUNKS)], F32, tag="accq")
        # column-sum accumulator over local coltiles
        colacc = singles.tile([128, KEYTILES], F32, tag="colacc")
        nc.vector.memset(colacc[:], 0.0)

        # ---- work schedule: (need_cols, t, m, cs, ch) ----
        work = []
        for t in range(QT):
            for m, (cs, ch) in enumerate(_chunks_for(t)):
                work.append((cs + ch, t, m, cs, ch))
        work.sort()
        wpos = [0]

        zcr = zc_ext.ap().rearrange("(c p) d -> p c d", p=128)
        eng_idx = [0]

        with tc.tile_pool(name="mpsum", bufs=2, space="PSUM") as mpsum, \
             tc.tile_pool(name="cpsum", bufs=1, space="PSUM") as cpsum, \
             tc.tile_pool(name="tpsum", bufs=1, space="PSUM") as tpsum:

            def main_chunk(t, m, cs, ch):
                mm = mpsum.tile([128, 1536], F32, tag="mp", name="mm")
                for j0 in range(0, ch, 512):
                    w = min(512, ch - j0)
                    nc.tensor.matmul(
                        mm[:, j0:j0 + w],
                        lhsT=znt8[:, :, t * 128:(t + 1) * 128],
                        rhs=znt8[:, :, cs + j0:cs + j0 + w],
                        start=True, stop=True, perf_mode=DR,
                        skip_group_check=True)
                texp = texps.tile([128, 1536], BF16, tag="texp", name="texp")
                nc.scalar.activation(out=texp[:, 0:ch], in_=mm[:, 0:ch],
                                     func=AF.Exp, scale=EXP_SCALE,
                                     accum_out=accq[:, t, m:m + 1])
                # colsums for shifts r in 1..63 within this chunk
                ntile = ch // 128
                r0 = cs // 128 - t          # shift of first tile in chunk
                lo = max(r0, 1)
                hi = min(r0 + ntile, 64)    # exclusive
                if hi <= lo:
                    return
                colp = cpsum.tile([128, 16], F32, tag="cp", name="colp")
                for r in range(lo, hi):
                    i = r - r0              # tile index within chunk
                    nc.tensor.matmul(
                        colp[:, i:i + 1],
                        lhsT=texp[:, i * 128:(i + 1) * 128],
                        rhs=ones[:], start=True, stop=True,
                        skip_group_check=True)
                # colacc[:, t+lo : t+hi] += colp[:, lo-r0 : hi-r0]
                # (DVE only: GPSIMD cannot access PSUM)
                nc.vector.tensor_add(colacc[:, t + lo:t + hi],
                                     colacc[:, t + lo:t + hi],
                                     colp[:, lo - r0:hi - r0])

            for k in range(NLC):
                A = LCH // 128
                zbig = loads.tile([128, A, D], F32, tag="zbig")
                nc.sync.dma_start(out=zbig[:], in_=zcr[:, k * A:(k + 1) * A, :])
                stats = small.tile([128, A, 6], F32, tag="stats")
                mv = small.tile([128, A, 2], F32, tag="mv")
                for a in range(A):
                    nc.vector.bn_stats(out=stats[:, a, :], in_=zbig[:, a, :])
                    nc.vector.bn_aggr(out=mv[:, a, :], in_=stats[:, a, :])
                m2 = small.tile([128, A], F32, tag="m2")
                nc.gpsimd.tensor_mul(m2[:], mv[:, :, 0], mv[:, :, 0])
                n2m = small.tile([128, A], F32, tag="n2m")
                nc.gpsimd.tensor_add(n2m[:], m2[:], mv[:, :, 1])
                lnn = small.tile([128, A], F32, tag="lnn")
                nc.scalar.activation(out=lnn[:], in_=n2m[:], func=AF.Ln)
                rn = small.tile([128, A], F32, tag="rn")
                # rn = SCALE_Q * exp(-0.5*ln(D*n2m)) folded into exp bias
                nc.scalar.activation(
                    out=rn[:], in_=lnn[:], func=AF.Exp, scale=-0.5,
                    bias=bias_const(math.log(SCALE_Q) - 0.5 * math.log(D)))
                znb = stage.tile([128, A, D], BF16, tag="znb")
                meng = nc.vector if k % 2 == 0 else nc.gpsimd
                for a in range(A):
                    meng.tensor_scalar_mul(znb[:, a, :], zbig[:, a, :],
                                           rn[:, a:a + 1])
                if k < 2:
                    # query chunks: exact ||4*q8||^2 via bn stats on the
                    # fp8-rounded values (round-trip bf16 -> fp8 -> bf16)
                    q8 = stage.tile([128, A, D], FP8, tag="q8")
                    nc.gpsimd.tensor_copy(q8[:], znb[:])
                    q8b = stage.tile([128, A, D], BF16, tag="q8b")
                    nc.gpsimd.tensor_copy(q8b[:], q8[:])
                    qstats = small.tile([128, A, 6], F32, tag="qstats")
                    qmv = small.tile([128, A, 2], F32, tag="qmv")
                    for a in range(A):
                        nc.vector.bn_stats(out=qstats[:, a, :], in_=q8b[:, a, :])
                        nc.vector.bn_aggr(out=qmv[:, a, :], in_=qstats[:, a, :])
                    qm2 = small.tile([128, A], F32, tag="qm2")
                    nc.gpsimd.tensor_mul(qm2[:], qmv[:, :, 0], qmv[:, :, 0])
                    nc.gpsimd.tensor_add(d2q[:, k * A:(k + 1) * A], qm2[:],
                                         qmv[:, :, 1])
                # transpose the A tiles (both k-halves; bf16) in two groups
                # of 4 through a dedicated 1-bank PSUM pool, casting to fp8
                # on the PSUM->SBUF copy
                for g in range(2):
                    pt = tpsum.tile([128, KH, 512], BF16, tag="tp", name="pt")
                    for h in range(KH):
                        for a in range(4):
                            nc.tensor.transpose(
                                pt[:, h, a * 128:(a + 1) * 128],
                                znb[:, 4 * g + a, h * 128:(h + 1) * 128],
                                ident[:])
                    nc.vector.tensor_copy(
                        znt8[:, :, k * LCH + g * 512:k * LCH + (g + 1) * 512],
                        pt[:])
                if k == 1:
                    nc.scalar.activation(out=expdq[:], in_=d2q[:],
                                         func=AF.Exp, scale=EXP_SCALE * D)
                # emit all main work whose columns are now resident
                avail = (k + 1) * LCH
                while wpos[0] < len(work) and work[wpos[0]][0] <= avail:
                    _, t, m, cs, ch = work[wpos[0]]
                    wpos[0] += 1
                    main_chunk(t, m, cs, ch)
            assert wpos[0] == len(work)

            # ---- positive pairs + supervised (tiny) ----
            _emit_pos_sup(nc, tc, cfg, zp1_ext, zp2_ext, sup_ext,
                          partials, loads, stage, small, bias_const)

        # ---- epilogue: row sums per owned rowtile minus self-sim ----
        rs = small.tile([128, QT], F32, tag="rs")
        nc.vector.tensor_reduce(out=rs[:], in_=accq[:],
                                axis=mybir.AxisListType.X,
                                op=mybir.AluOpType.add)
        rowadj = small.tile([128, QT], F32, tag="rowadj")
        nc.vector.tensor_sub(rowadj[:], rs[:], expdq[:])
        nc.sync.dma_start(out=rowadj_ext.ap(), in_=rowadj[:])
        nc.sync.dma_start(out=colacc_ext.ap(), in_=colacc[:])
        nc.sync.dma_start(out=part_ext.ap(), in_=partials[:])


def _emit_pos_sup(nc, tc, cfg, zp1_ext, zp2_ext, sup_ext, partials,
                  loads, stage, small, bias_const):
    A = cfg.pa
    zp1r = zp1_ext.ap().rearrange("(a p) d -> p a d", p=128)
    zp2r = zp2_ext.ap().rearrange("(a p) d -> p a d", p=128)
    p1 = loads.tile([128, A, D], F32, tag="p1", bufs=1)
    p2 = loads.tile([128, A, D], F32, tag="p2", bufs=1)
    nc.sync.dma_start(out=p1[:], in_=zp1r)
    nc.sync.dma_start(out=p2[:], in_=zp2r)
    prod = stage.tile([128, A, D], F32, tag="prod", bufs=1)
    nc.gpsimd.tensor_mul(prod[:], p1[:], p2[:])
    stats = small.tile([128, 3, A, 6], F32, tag="pstats")
    mv = small.tile([128, 3, A, 2], F32, tag="pmv")
    for i, src in enumerate((p1, p2, prod)):
        for a in range(A):
            nc.vector.bn_stats(out=stats[:, i, a, :], in_=src[:, a, :])
            nc.vector.bn_aggr(out=mv[:, i, a, :], in_=stats[:, i, a, :])
    m2 = small.tile([128, 2, A], F32, tag="pm2")
    nc.vector.tensor_mul(m2[:], mv[:, 0:2, :, 0], mv[:, 0:2, :, 0])
    n2ab = small.tile([128, 2, A], F32, tag="n2ab")
    nc.vector.tensor_add(n2ab[:], m2[:], mv[:, 0:2, :, 1])
    lnab = small.tile([128, 2, A], F32, tag="lnab")
    nc.scalar.activation(out=lnab[:], in_=n2ab[:], func=AF.Ln)
    lnsum = small.tile([128, A], F32, tag="lnsum")
    nc.vector.tensor_add(lnsum[:], lnab[:, 0, :], lnab[:, 1, :])
    rp = small.tile([128, A], F32, tag="rp")
    nc.scalar.activation(out=rp[:], in_=lnsum[:], func=AF.Exp,
                         scale=-0.5, bias=bias_const(-math.log(D)))
    pos = small.tile([128, A], F32, tag="pos")
    nc.vector.tensor_mul(pos[:], mv[:, 2, :, 0], rp[:])
    pdump = small.tile([128, A], F32, tag="pdump")
    nc.scalar.activation(out=pdump[:], in_=pos[:], func=AF.Identity,
                         scale=float(D), accum_out=partials[:, 1:2])

    S = cfg.sa
    supr = sup_ext.ap().rearrange("s (p a) -> p s a", p=128)
    sup = loads.tile([128, 6, S], F32, tag="sup", bufs=1)
    nc.sync.dma_start(out=sup[:], in_=supr)
    d8 = small.tile([128, S], F32, tag="d8")
    sdump = small.tile([128, S], F32, tag="sdump")
    nc.vector.tensor_sub(d8[:], sup[:, 0, :], sup[:, 1, :])
    nc.scalar.activation(out=sdump[:], in_=d8[:], func=AF.Square,
                         accum_out=partials[:, 2:3])
    d8b = small.tile([128, S], F32, tag="d8b")
    nc.vector.tensor_sub(d8b[:], sup[:, 2, :], sup[:, 3, :])
    nc.scalar.activation(out=sdump[:], in_=d8b[:], func=AF.Square,
                         accum_out=partials[:, 3:4])
    x_ap = sup[:, 4, :]
    t_ap = sup[:, 5, :]
    r8 = small.tile([128, S], F32, tag="r8")
    nc.scalar.activation(out=r8[:], in_=x_ap, func=AF.Relu)
    a8 = small.tile([128, S], F32, tag="a8")
    nc.scalar.activation(out=a8[:], in_=x_ap, func=AF.Abs)
    e8 = small.tile([128, S], F32, tag="e8")
    nc.scalar.activation(out=e8[:], in_=a8[:], func=AF.Exp, scale=-1.0)
    l8 = small.tile([128, S], F32, tag="l8")
    nc.scalar.activation(out=l8[:], in_=e8[:], func=AF.Ln, bias=1.0)
    xt8 = small.tile([128, S], F32, tag="xt8")
    nc.vector.tensor_mul(xt8[:], x_ap, t_ap)
    s1 = small.tile([128, S], F32, tag="s1")
    nc.vector.tensor_add(s1[:], r8[:], l8[:])
    s2 = small.tile([128, S], F32, tag="s2")
    nc.vector.tensor_sub(s2[:], s1[:], xt8[:])
    nc.scalar.activation(out=sdump[:], in_=s2[:], func=AF.Identity,
                         accum_out=partials[:, 4:5])


def make_in_maps(cfg, price_pred, price_target, change_pred, change_target,
                 criticality_pred, criticality_target, z1, z2):
    z1 = np.asarray(z1, dtype=np.float32)
    z2 = np.asarray(z2, dtype=np.float32)
    zfull = np.concatenate([z1, z2], axis=0)
    sups = [np.asarray(a, dtype=np.float32).reshape(-1) for a in
            (price_pred, price_target, change_pred, change_target,
             criticality_pred, criticality_target)]
    in_maps = []
    pr = cfg.pos_rows
    for c in range(N_CORES):
        rot = c * (TWO_N // N_CORES)
        idx = (rot + np.arange(KEYROWS)) % TWO_N
        zcat = np.ascontiguousarray(zfull[idx])
        sl = slice(c * pr, (c + 1) * pr)
        sup = np.stack([s[c * cfg.sup_rows:(c + 1) * cfg.sup_rows]
                        for s in sups])
        in_maps.append({
            "zcat": zcat,
            "zp1": np.ascontiguousarray(z1[sl]),
            "zp2": np.ascontiguousarray(z2[sl]),
            "sup": np.ascontiguousarray(sup),
        })
    return in_maps


def combine(cfg, results):
    # per-row denominator assembly: rowfull[p, T] over global rowtiles T
    rowfull = np.zeros((128, NT), np.float64)
    colsum = np.zeros(8, np.float64)
    for c, r in enumerate(results):
        rowadj = r["rowadj"].astype(np.float64)       # [128, 16]
        base = c * QT
        rowfull[:, base:base + QT] += rowadj
        cacc = r["colacc"].astype(np.float64)         # [128, 80]
        for j in range(1, KEYTILES):
            rowfull[:, (base + j) % NT] += cacc[:, j]
        colsum += r["partials"].astype(np.float64).sum(axis=0)
    slog = float(np.log(rowfull).sum())
    sdot, sprice, schange, scrit = colsum[1], colsum[2], colsum[3], colsum[4]
    n = float(cfg.n)
    ssl = (slog - 2.0 * ITEMP * sdot) / (2.0 * n)
    supervised = (W_PRICE * sprice + W_CHANGE * schange + W_CRIT * scrit) / n
    return np.float32(supervised + SSL_WEIGHT * ssl)


_compiled = {}


def _get_program(repeat=1):
    key = repeat
    if key not in _compiled:
        _compiled[key] = build_program(FULL, repeat=repeat)
    return _compiled[key]


def kernel(**inputs):
    nc = _get_program()
    in_maps = make_in_maps(FULL, **inputs)
    res = run_bass_kernel_spmd(nc, in_maps, list(range(N_CORES)))
    return combine(FULL, res.results)
